# revision 11
# baseline (speedup 1.0000x reference)
"""Trainium2 Bass kernel for nn_CrossAttention_249108103802.

8 cores data-parallel over B=8; per core (batch b):
  G_s   = x_s^T x_s            (Gram, fp16 operands, fp32 psum, upper tri)
  A_s   = (G_s - mu I) Wv_s    (fp16)
  ctp_s = A_s^T Wk_s + mu Wv_s^T Wk_s   (fp16 pair-packed + fp32 TT)
  ctx_s = softmax_d(scale * ctp_s)      (per-head 64x64)
  o2^T  = blockdiag(ctx1) stationary @ xT2   fused into x2 streaming
  o1^T  = same with ctx2 / resident xT1

The PE p-state model rewards gap-free streams (half clock for 3us after
any idle), so the softmax tails are emitted as staged generators with
matmul filler interleaved between stages: phase-2 gram tiles fill
ctx_tail(1)'s stages, deferred o2 out-matmuls + per-head-pair o1
matmuls fill ctx_tail(2)'s. Host supplies x and W in fp16; xT is built
on-chip with PE transposes. Outputs are written as o^T [C, N] fp16 and
transposed back on the host.
"""
import sys

sys.path.insert(0, "/opt/trn_rl_repo")

import numpy as np

import concourse.bass as bass
import concourse.mybir as mybir
import concourse.tile as tile
from concourse import bacc
from concourse.bass_utils import run_bass_kernel_spmd
from concourse.masks import make_identity

B, N, C, H = 8, 4096, 512, 8
HD = C // H                    # 64
SCALE = HD ** -0.5             # 1/8
MU = float(N)
NT = N // 128                  # 32 row tiles
CB = C // 128                  # 4 feature blocks
HP = H // 2                    # 4 head pairs
NG = 8                         # streaming groups
GW = N // NG                   # 512 rows/cols per group
TPG = NT // NG                 # 4 row tiles per group
DEFER = 3                      # o2 groups deferred into ctx_tail(2)
F16 = mybir.dt.float16
F32 = mybir.dt.float32
AF = mybir.ActivationFunctionType

# Gram psum column ranges per row-block m (strict upper triangle)
GCOL = [(0, 512), (128, 512), (256, 512), (384, 512)]
# lower-triangle tiles needing a PE transpose
LOWT = [(1, 0), (2, 0), (2, 1), (3, 0), (3, 1), (3, 2)]


def build():
    nc = bacc.Bacc("TRN2", target_bir_lowering=False, debug=False, num_devices=8)
    x_d = [nc.declare_dram_parameter(f"x{s + 1}", [N, C], F16, isOutput=False)
           for s in range(2)]
    w_d = [nc.declare_dram_parameter(f"w{s + 1}", [C, 2 * C], F16, isOutput=False)
           for s in range(2)]
    o_d = [nc.declare_dram_parameter(f"ot{s + 1}", [C, N], F16, isOutput=True)
           for s in range(2)]

    with tile.TileContext(nc) as tc:
        with (
            tc.tile_pool(name="const", bufs=1) as constp,
            tc.tile_pool(name="wf", bufs=1) as wfp,
            tc.tile_pool(name="tts", bufs=1) as ttsp,
            tc.tile_pool(name="x", bufs=6) as xp,
            tc.tile_pool(name="xt", bufs=1) as xtp,
            tc.tile_pool(name="g", bufs=1) as gp_,
            tc.tile_pool(name="a", bufs=1) as ap_,
            tc.tile_pool(name="cx", bufs=1) as cxp,
            tc.tile_pool(name="osb", bufs=4) as osp,
            tc.tile_pool(name="ob1", bufs=NG) as ob1p,
            tc.tile_pool(name="ps_g", bufs=1, space="PSUM") as psg,
            tc.tile_pool(name="ps_t", bufs=2, space="PSUM") as pst,
            tc.tile_pool(name="ps_o", bufs=2, space="PSUM") as pso,
        ):
            identf = constp.tile([128, 128], F32, tag="identf")
            make_identity(nc, identf[:])
            ident16 = constp.tile([128, 128], F16, tag="ident16")
            nc.scalar.copy(ident16[:], identf[:])
            muI = constp.tile([128, 128], F32, tag="muI")
            nc.gpsimd.memset(muI[:], 0.0)
            nc.gpsimd.affine_select(
                out=muI[:], in_=muI[:],
                compare_op=mybir.AluOpType.not_equal, fill=MU,
                base=0, pattern=[[-1, 128]], channel_multiplier=1,
            )

            # ---- weight loads (scalar HWDGE queue) ----
            wfs, ttss = [], []
            for s in range(2):
                wf = wfp.tile([128, CB, 2 * C], F16, tag=f"wf{s}")
                nc.scalar.dma_start(
                    out=wf[:], in_=w_d[s][:, :].rearrange("(a p) m -> p a m", p=128))
                wfs.append(wf)
                ttss.append(ttsp.tile([128, HP, 128], F32, tag=f"tts{s}",
                                      name=f"tts{s}"))

            # ---- x streaming loads (sync HWDGE queue); first group split
            # in half so the very first gram matmul starts sooner ----
            xcs = {}
            for s in range(2):
                for g in range(NG):
                    xc = xp.tile([128, TPG, C], F16, tag="xc", name=f"xc{s}_{g}")
                    if s == 0 and g == 0:
                        for h in range(2):
                            nc.sync.dma_start(
                                out=xc[:, 2 * h:2 * h + 2, :],
                                in_=x_d[0][256 * h:256 * (h + 1), :].rearrange(
                                    "(t p) c -> p t c", p=128))
                    else:
                        nc.sync.dma_start(
                            out=xc[:],
                            in_=x_d[s][GW * g:GW * (g + 1), :].rearrange(
                                "(t p) c -> p t c", p=128))
                    xcs[(s, g)] = xc

            xts = [xtp.tile([128, CB, N], F16, tag=f"xt{s}", name=f"xt{s}")
                   for s in range(2)]
            eng = [nc.vector.tensor_copy, nc.scalar.copy]

            def tt_weights(s):
                # exact TT = mu * Wv^T Wk, pair-packed [e(2h), d(2h)]
                wf = wfs[s]
                for hp in range(HP):
                    ttp = pso.tile([128, C], F32, tag="op", name=f"ttp{s}_{hp}")
                    for a in range(CB):
                        nc.tensor.matmul(
                            ttp[:, 0:128],
                            lhsT=wf[:, a, C + 128 * hp:C + 128 * (hp + 1)],
                            rhs=wf[:, a, 128 * hp:128 * (hp + 1)],
                            start=(a == 0), stop=(a == CB - 1))
                    nc.scalar.mul(ttss[s][:, hp, :], ttp[:, 0:128], MU)

            def gram_mm(s, t, gps):
                xc = xcs[(s, t // TPG)]
                tt_ = t % TPG
                for m in range(CB):
                    lo, hi = GCOL[m]
                    nc.tensor.matmul(
                        gps[m][:],
                        lhsT=xc[:, tt_, 128 * m:128 * (m + 1)],
                        rhs=xc[:, tt_, lo:hi],
                        start=(t == 0), stop=(t == NT - 1))

            def xpose(s, t):
                """PE-transpose the 4 column blocks of x tile t into xT (f16)."""
                xc = xcs[(s, t // TPG)]
                tt_ = t % TPG
                tp4 = pst.tile([128, CB, 128], F16, tag="tp", name=f"tp4_{s}_{t}")
                for cb in range(CB):
                    nc.tensor.transpose(
                        tp4[:, cb, :], xc[:, tt_, 128 * cb:128 * (cb + 1)],
                        ident16[:])
                eng[t % 2](xts[s][:, :, 128 * t:128 * (t + 1)], tp4[:])

            def ctx_tail_gen(s, gps, on_hp=None):
                """Staged G->A->ctp->softmax->cbd; yields between stages so
                the caller can interleave PE filler work."""
                wf = wfs[s]
                gsb = gp_.tile([128, CB, C], F16, tag="gsb", name=f"gsb{s}")
                for m in range(CB):
                    lo, hi = GCOL[m]
                    dg = 128 * m - lo   # diag block offset inside psum tile
                    nc.vector.tensor_sub(
                        gsb[:, m, 128 * m:128 * (m + 1)],
                        gps[m][:, dg:dg + 128], muI[:])
                    if m < 3:
                        eng[m % 2](gsb[:, m, 128 * (m + 1):C],
                                   gps[m][:, dg + 128:hi - lo])
                esb = cxp.tile([128, HP, 128], F32, tag="esb", name=f"esb{s}")
                ssum = cxp.tile([128, HP], F32, tag="ssum", name=f"ssum{s}")
                rsum = cxp.tile([128, HP], F32, tag="rsum", name=f"rsum{s}")
                comb = cxp.tile([128, HP, 128], F32, tag="comb", name=f"comb{s}")
                ctxts = cxp.tile([128, HP, 128], F16, tag="ctxts",
                                 name=f"ctxts{s}")
                nc.gpsimd.memset(ctxts[:], 0.0)
                yield
                # lower-triangle tiles by PE transpose (f16)
                gtr = gp_.tile([128, len(LOWT), 128], F16, tag="gtr",
                               name=f"gtr{s}")
                for i, (a2, b2) in enumerate(LOWT):
                    tpg = pst.tile([128, CB, 128], F16, tag="tp",
                                   name=f"tpg{s}_{i}")
                    nc.tensor.transpose(
                        tpg[:, 0, :], gsb[:, b2, 128 * a2:128 * (a2 + 1)],
                        ident16[:])
                    nc.vector.tensor_copy(gtr[:, i, :], tpg[:, 0, :])
                low = {ab_: i for i, ab_ in enumerate(LOWT)}
                yield

                def g_tile(a2, b2):
                    if b2 >= a2:
                        return gsb[:, a2, 128 * b2:128 * (b2 + 1)]
                    return gtr[:, low[(a2, b2)], :]

                # A = Gc^T-tiles @ Wv (f16, free 512)
                ab = ap_.tile([128, CB, C], F16, tag="ab", name=f"ab{s}")
                for b2 in range(CB):
                    apx = pso.tile([128, C], F32, tag="op", name=f"apx{s}_{b2}")
                    for a2 in range(CB):
                        nc.tensor.matmul(
                            apx[:], lhsT=g_tile(a2, b2), rhs=wf[:, a2, C:2 * C],
                            start=(a2 == 0), stop=(a2 == CB - 1))
                    eng[b2 % 2](ab[:, b2, :], apx[:])
                    if b2 == 1:
                        yield
                yield
                # ctp (pair-packed) + TT, exp halves, per-hp reciprocal
                for hp in range(HP):
                    ctp = pso.tile([128, C], F32, tag="op", name=f"ctp{s}_{hp}")
                    sl = slice(128 * hp, 128 * (hp + 1))
                    for b2 in range(CB):
                        nc.tensor.matmul(
                            ctp[:, 0:128], lhsT=ab[:, b2, sl], rhs=wf[:, b2, sl],
                            start=(b2 == 0), stop=(b2 == CB - 1))
                    nc.vector.tensor_add(comb[:, hp, :], ctp[:, 0:128],
                                         ttss[s][:, hp, :])
                    nc.scalar.activation(
                        esb[0:64, hp, 0:64], comb[0:64, hp, 0:64], AF.Exp,
                        scale=SCALE, accum_out=ssum[0:64, hp:hp + 1])
                    nc.scalar.activation(
                        esb[64:128, hp, 64:128], comb[64:128, hp, 64:128], AF.Exp,
                        scale=SCALE, accum_out=ssum[64:128, hp:hp + 1])
                    nc.vector.reciprocal(rsum[:, hp:hp + 1], ssum[:, hp:hp + 1])
                    if hp % 2 == 1:
                        yield
                cbd = cxp.tile([128, HP, 128], F16, tag=f"cbd{s}")
                for hp in range(HP):
                    nc.vector.tensor_scalar_mul(
                        ctxts[0:64, hp, 0:64], esb[0:64, hp, 0:64],
                        rsum[0:64, hp:hp + 1])
                    nc.vector.tensor_scalar_mul(
                        ctxts[64:128, hp, 64:128], esb[64:128, hp, 64:128],
                        rsum[64:128, hp:hp + 1])
                    tpc = pst.tile([128, CB, 128], F16, tag="tp",
                                   name=f"tpc{s}_{hp}")
                    nc.tensor.transpose(tpc[:, 0, :], ctxts[:, hp, :],
                                        ident16[:])
                    nc.scalar.copy(cbd[:, hp, :], tpc[:, 0, :])
                    if on_hp is not None:
                        on_hp(hp, cbd)
                    yield
                ctx_res[s] = cbd

            ctx_res = {}

            def out_mm(s, cb, g, cbd, ob):
                """o_s^T[cb block, group g] = cbd[cb] stationary @ xT."""
                op = pso.tile([128, C], F32, tag="op", name=f"op{s}_{cb}_{g}")
                nc.tensor.matmul(
                    op[:], lhsT=cbd[:, cb, :],
                    rhs=xts[s][:, cb, GW * g:GW * (g + 1)],
                    start=True, stop=True)
                eng[(cb + g) % 2](ob[:, cb, :], op[:])

            def store_o2(g, ob):
                nc.scalar.dma_start(
                    out=o_d[1][:, GW * g:GW * (g + 1)].rearrange(
                        "(a p) n -> p a n", p=128),
                    in_=ob[:])

            # ================= phase 1: x1 gram + xpose =================
            gps1 = [psg.tile([128, hi - lo], F32, tag=f"gp{m}", name=f"gp{m}_0")
                    for m, (lo, hi) in enumerate(GCOL)]
            for t in range(NT):
                gram_mm(0, t, gps1)
                xpose(0, t)
                if t == 3:
                    tt_weights(0)
                if t == 7:
                    tt_weights(1)

            # ========== phase 2: x2 gram + xpose + fused o2^T,  ==========
            # ========== interleaved with ctx_tail(1)'s stages   ==========
            gps2 = [psg.tile([128, hi - lo], F32, tag=f"gp{m}", name=f"gp{m}_1")
                    for m, (lo, hi) in enumerate(GCOL)]
            tail1 = ctx_tail_gen(0, gps1)
            next(tail1)          # emit G1 copies first (frees gram psum banks)
            t2 = 0               # phase-2 tile cursor

            def emit_tiles(k):
                nonlocal t2
                for _ in range(k):
                    if t2 >= NT:
                        return
                    gram_mm(1, t2, gps2)
                    xpose(1, t2)
                    t2 += 1

            while True:
                emit_tiles(2)
                try:
                    next(tail1)
                except StopIteration:
                    break
            # drain remaining x2 tiles, then o2 groups 0..NG-DEFER-1
            emit_tiles(NT)
            obs2 = {}
            for g in range(NG - DEFER):
                ob = osp.tile([128, CB, GW], F16, tag="ob", name=f"ob2_{g}")
                for cb in range(CB):
                    out_mm(1, cb, g, ctx_res[0], ob)
                store_o2(g, ob)

            # ==== tail(2) + phase 3, with deferred o2 groups as filler ====
            ob1s = [ob1p.tile([128, CB, GW], F16, tag="ob1", name=f"ob1_{g}")
                    for g in range(NG)]
            defer_q = [(cb, g) for g in range(NG - DEFER, NG) for cb in range(CB)]
            for g in range(NG - DEFER, NG):
                obs2[g] = osp.tile([128, CB, GW], F16, tag="ob",
                                   name=f"ob2_{g}")
            di = 0

            def emit_defer(k):
                nonlocal di
                for _ in range(k):
                    if di >= len(defer_q):
                        return
                    cb, g = defer_q[di]
                    out_mm(1, cb, g, ctx_res[0], obs2[g])
                    if cb == CB - 1:
                        store_o2(g, obs2[g])
                    di += 1

            def phase3(hp, cbd):
                for g in range(NG):
                    out_mm(0, hp, g, cbd, ob1s[g])
                if hp % 2 == 1:
                    h = hp // 2
                    for g in range(NG):
                        nc.sync.dma_start(
                            out=o_d[0][256 * h:256 * (h + 1),
                                       GW * g:GW * (g + 1)].rearrange(
                                "(a p) n -> p a n", p=128),
                            in_=ob1s[g][:, 2 * h:2 * h + 2, :])

            tail2 = ctx_tail_gen(1, gps2, on_hp=phase3)
            next(tail2)
            while True:
                emit_defer(2)
                try:
                    next(tail2)
                except StopIteration:
                    break
            emit_defer(len(defer_q))
    nc.compile()
    return nc


_NC = None


def make_in_maps(inputs):
    x1 = np.asarray(inputs["x1"])
    x2 = np.asarray(inputs["x2"])
    w1 = np.ascontiguousarray(np.asarray(inputs["W_kv1"]), dtype=np.float16)
    w2 = np.ascontiguousarray(np.asarray(inputs["W_kv2"]), dtype=np.float16)
    in_maps = []
    for b in range(B):
        in_maps.append({
            "x1": np.ascontiguousarray(x1[b], dtype=np.float16),
            "x2": np.ascontiguousarray(x2[b], dtype=np.float16),
            "w1": w1, "w2": w2,
        })
    return in_maps


def kernel(x1, x2, W_kv1, W_kv2):
    global _NC
    if _NC is None:
        _NC = build()
    in_maps = make_in_maps(
        {"x1": x1, "x2": x2, "W_kv1": W_kv1, "W_kv2": W_kv2})
    res = run_bass_kernel_spmd(_NC, in_maps, core_ids=list(range(B)))
    o1 = np.stack([res.results[b]["ot1"].astype(np.float32).T
                   for b in range(B)])
    o2 = np.stack([res.results[b]["ot2"].astype(np.float32).T
                   for b in range(B)])
    return o1, o2


# revision 21
# speedup vs baseline: 1.0960x; 1.0960x over previous
"""Trainium2 Bass kernel for nn_CrossAttention_249108103802.

8 cores data-parallel over B=8; per core (batch b):
  G_s   = x_s^T x_s            (Gram, fp16 operands, fp32 psum, upper tri)
  A_s   = (G_s - mu I) Wv_s    (fp16)
  ctp_s = A_s^T Wk_s + mu Wv_s^T Wk_s   (fp16 pair-packed + fp32 TT)
  ctx_s = softmax_d(scale * ctp_s)      (per-head 64x64)
  o2^T  = blockdiag(ctx1) stationary @ xT2   fused into x2 streaming
  o1^T  = same with ctx2 / resident xT1

The PE p-state model rewards gap-free streams (half clock for 3us after
any idle), so the softmax tails are emitted as staged generators with
matmul filler interleaved between stages: phase-2 gram tiles fill
ctx_tail(1)'s stages, deferred o2 out-matmuls + per-head-pair o1
matmuls fill ctx_tail(2)'s. Host supplies x and W in fp16; xT is built
on-chip with PE transposes. Outputs are written as o^T [C, N] fp16 and
transposed back on the host.
"""
import sys

sys.path.insert(0, "/opt/trn_rl_repo")

import numpy as np

import concourse.bass as bass
import concourse.mybir as mybir
import concourse.tile as tile
from concourse import bacc
from concourse.bass_utils import run_bass_kernel_spmd
from concourse.masks import make_identity

B, N, C, H = 8, 4096, 512, 8
HD = C // H                    # 64
SCALE = HD ** -0.5             # 1/8
MU = float(N)
NT = N // 128                  # 32 row tiles
CB = C // 128                  # 4 feature blocks
HP = H // 2                    # 4 head pairs
NG = 8                         # streaming groups
GW = N // NG                   # 512 rows/cols per group
TPG = NT // NG                 # 4 row tiles per group
DEFER = 3                      # o2 groups deferred into ctx_tail(2)
F16 = mybir.dt.float16
F32 = mybir.dt.float32
AF = mybir.ActivationFunctionType

# Gram psum column ranges per row-block m (strict upper triangle)
GCOL = [(0, 512), (128, 512), (256, 512), (384, 512)]
# column offset of each m's accumulator inside the packed 3-bank psum
# tile: m1 (384 cols) and m3 (128 cols) share bank 1. m1 owns the bank's
# start (first write at t=0) and stop (last write at t=NT-1); m3 always
# runs with start=stop=False, relying on the bank's pending-zero bytes.
GOFF = [0, 512, 1024, 896]
GPW = 1536
# lower-triangle tiles needing a PE transpose
LOWT = [(1, 0), (2, 0), (2, 1), (3, 0), (3, 1), (3, 2)]


def build():
    nc = bacc.Bacc("TRN2", target_bir_lowering=False, debug=False, num_devices=8)
    x_d = [nc.declare_dram_parameter(f"x{s + 1}", [N, C], F16, isOutput=False)
           for s in range(2)]
    w_d = [nc.declare_dram_parameter(f"w{s + 1}", [C, 2 * C], F16, isOutput=False)
           for s in range(2)]
    o_d = [nc.declare_dram_parameter(f"ot{s + 1}", [C, N], F16, isOutput=True)
           for s in range(2)]

    with tile.TileContext(nc) as tc:
        with (
            tc.tile_pool(name="const", bufs=1) as constp,
            tc.tile_pool(name="wf", bufs=1) as wfp,
            tc.tile_pool(name="tts", bufs=1) as ttsp,
            tc.tile_pool(name="x", bufs=6) as xp,
            tc.tile_pool(name="xt", bufs=1) as xtp,
            tc.tile_pool(name="g", bufs=1) as gp_,
            tc.tile_pool(name="a", bufs=1) as ap_,
            tc.tile_pool(name="cx", bufs=1) as cxp,
            tc.tile_pool(name="osb", bufs=4) as osp,
            tc.tile_pool(name="ob1", bufs=NG) as ob1p,
            tc.tile_pool(name="ps_g", bufs=1, space="PSUM") as psg,
            tc.tile_pool(name="ps_t", bufs=2, space="PSUM") as pst,
            tc.tile_pool(name="ps_o", bufs=3, space="PSUM") as pso,
        ):
            identf = constp.tile([128, 128], F32, tag="identf")
            make_identity(nc, identf[:])
            ident16 = constp.tile([128, 128], F16, tag="ident16")
            nc.scalar.copy(ident16[:], identf[:])
            muI = constp.tile([128, 128], F32, tag="muI")
            nc.gpsimd.memset(muI[:], 0.0)
            nc.gpsimd.affine_select(
                out=muI[:], in_=muI[:],
                compare_op=mybir.AluOpType.not_equal, fill=MU,
                base=0, pattern=[[-1, 128]], channel_multiplier=1,
            )

            # ---- x1 streaming loads first (sync HWDGE queue; the DMA pipe
            # is serialized, so issue order is transfer order). First group
            # split in half so the very first gram matmul starts sooner ----
            xcs = {}
            for g in range(NG):
                xc = xp.tile([128, TPG, C], F16, tag="xc", name=f"xc0_{g}")
                if g == 0:
                    for h in range(2):
                        nc.sync.dma_start(
                            out=xc[:, 2 * h:2 * h + 2, :],
                            in_=x_d[0][256 * h:256 * (h + 1), :].rearrange(
                                "(t p) c -> p t c", p=128))
                else:
                    nc.sync.dma_start(
                        out=xc[:],
                        in_=x_d[0][GW * g:GW * (g + 1), :].rearrange(
                            "(t p) c -> p t c", p=128))
                xcs[(0, g)] = xc

            # ---- weight loads (scalar HWDGE queue), after x1 ----
            wfs, ttss = [], []
            for s in range(2):
                wf = wfp.tile([128, CB, 2 * C], F16, tag=f"wf{s}")
                nc.scalar.dma_start(
                    out=wf[:], in_=w_d[s][:, :].rearrange("(a p) m -> p a m", p=128))
                wfs.append(wf)
                ttss.append(ttsp.tile([128, HP, 128], F32, tag=f"tts{s}",
                                      name=f"tts{s}"))

            # ---- x2 streaming loads ----
            for g in range(NG):
                xc = xp.tile([128, TPG, C], F16, tag="xc", name=f"xc1_{g}")
                nc.sync.dma_start(
                    out=xc[:],
                    in_=x_d[1][GW * g:GW * (g + 1), :].rearrange(
                        "(t p) c -> p t c", p=128))
                xcs[(1, g)] = xc

            xts = [xtp.tile([128, CB, N], F16, tag=f"xt{s}", name=f"xt{s}")
                   for s in range(2)]
            eng = [nc.vector.tensor_copy, nc.scalar.copy]

            def tt_weights(s):
                # exact TT = mu * Wv^T Wk, pair-packed [e(2h), d(2h)]
                wf = wfs[s]
                for hp in range(HP):
                    ttp = pso.tile([128, C], F32, tag="op", name=f"ttp{s}_{hp}")
                    for a in range(CB):
                        nc.tensor.matmul(
                            ttp[:, 0:128],
                            lhsT=wf[:, a, C + 128 * hp:C + 128 * (hp + 1)],
                            rhs=wf[:, a, 128 * hp:128 * (hp + 1)],
                            start=(a == 0), stop=(a == CB - 1))
                    nc.scalar.mul(ttss[s][:, hp, :], ttp[:, 0:128], MU)

            def gram_mm(s, t, gp):
                xc = xcs[(s, t // TPG)]
                tt_ = t % TPG
                order = [3, 1, 0, 2] if t == NT - 1 else [1, 3, 0, 2]
                for m in order:
                    lo, hi = GCOL[m]
                    nc.tensor.matmul(
                        gp[:, GOFF[m]:GOFF[m] + hi - lo],
                        lhsT=xc[:, tt_, 128 * m:128 * (m + 1)],
                        rhs=xc[:, tt_, lo:hi],
                        start=(t == 0 and m != 3),
                        stop=(t == NT - 1 and m != 3),
                        skip_group_check=(m == 3))

            def xpose(s, t):
                """PE-transpose the 4 column blocks of x tile t into xT (f16)."""
                xc = xcs[(s, t // TPG)]
                tt_ = t % TPG
                tp4 = pst.tile([128, CB, 128], F16, tag="tp", name=f"tp4_{s}_{t}")
                for cb in range(CB):
                    nc.tensor.transpose(
                        tp4[:, cb, :], xc[:, tt_, 128 * cb:128 * (cb + 1)],
                        ident16[:])
                eng[t % 2](xts[s][:, :, 128 * t:128 * (t + 1)], tp4[:])

            def ctx_tail_gen(s, gp, on_hp=None):
                """Staged G->A->ctp->softmax->cbd; yields between stages so
                the caller can interleave PE filler work."""
                wf = wfs[s]
                gsb = gp_.tile([128, CB, C], F16, tag="gsb", name=f"gsb{s}")
                for m in range(CB):
                    lo, hi = GCOL[m]
                    dg = GOFF[m] + 128 * m - lo  # diag offset in packed psum
                    nc.vector.tensor_sub(
                        gsb[:, m, 128 * m:128 * (m + 1)],
                        gp[:, dg:dg + 128], muI[:])
                    if m < 3:
                        eng[m % 2](gsb[:, m, 128 * (m + 1):C],
                                   gp[:, dg + 128:GOFF[m] + hi - lo])
                esb = cxp.tile([128, HP, 128], F32, tag="esb", name=f"esb{s}")
                ssum = cxp.tile([128, HP], F32, tag="ssum", name=f"ssum{s}")
                rsum = cxp.tile([128, HP], F32, tag="rsum", name=f"rsum{s}")
                comb = cxp.tile([128, HP, 128], F32, tag="comb", name=f"comb{s}")
                ctxts = cxp.tile([128, HP, 128], F16, tag="ctxts",
                                 name=f"ctxts{s}")
                nc.gpsimd.memset(ctxts[:], 0.0)
                yield
                # lower-triangle tiles by PE transpose (f16)
                gtr = gp_.tile([128, len(LOWT), 128], F16, tag="gtr",
                               name=f"gtr{s}")
                for i, (a2, b2) in enumerate(LOWT):
                    tpg = pst.tile([128, CB, 128], F16, tag="tp",
                                   name=f"tpg{s}_{i}")
                    nc.tensor.transpose(
                        tpg[:, 0, :], gsb[:, b2, 128 * a2:128 * (a2 + 1)],
                        ident16[:])
                    nc.vector.tensor_copy(gtr[:, i, :], tpg[:, 0, :])
                low = {ab_: i for i, ab_ in enumerate(LOWT)}
                yield

                def g_tile(a2, b2):
                    if b2 >= a2:
                        return gsb[:, a2, 128 * b2:128 * (b2 + 1)]
                    return gtr[:, low[(a2, b2)], :]

                # A = Gc^T-tiles @ Wv (f16, free 512)
                ab = ap_.tile([128, CB, C], F16, tag="ab", name=f"ab{s}")
                for b2 in range(CB):
                    apx = pso.tile([128, C], F32, tag="op", name=f"apx{s}_{b2}")
                    for a2 in range(CB):
                        nc.tensor.matmul(
                            apx[:], lhsT=g_tile(a2, b2), rhs=wf[:, a2, C:2 * C],
                            start=(a2 == 0), stop=(a2 == CB - 1))
                    eng[b2 % 2](ab[:, b2, :], apx[:])
                    if b2 == 1:
                        yield
                yield
                # ctp (pair-packed) + TT, exp halves, per-hp reciprocal
                for hp in range(HP):
                    ctp = pso.tile([128, C], F32, tag="op", name=f"ctp{s}_{hp}")
                    sl = slice(128 * hp, 128 * (hp + 1))
                    for b2 in range(CB):
                        nc.tensor.matmul(
                            ctp[:, 0:128], lhsT=ab[:, b2, sl], rhs=wf[:, b2, sl],
                            start=(b2 == 0), stop=(b2 == CB - 1))
                    nc.vector.tensor_add(comb[:, hp, :], ctp[:, 0:128],
                                         ttss[s][:, hp, :])
                    nc.scalar.activation(
                        esb[0:64, hp, 0:64], comb[0:64, hp, 0:64], AF.Exp,
                        scale=SCALE, accum_out=ssum[0:64, hp:hp + 1])
                    nc.scalar.activation(
                        esb[64:128, hp, 64:128], comb[64:128, hp, 64:128], AF.Exp,
                        scale=SCALE, accum_out=ssum[64:128, hp:hp + 1])
                    nc.vector.reciprocal(rsum[:, hp:hp + 1], ssum[:, hp:hp + 1])
                    if hp % 2 == 1:
                        yield
                cbd = cxp.tile([128, HP, 128], F16, tag=f"cbd{s}")
                for hp in range(HP):
                    nc.vector.tensor_scalar_mul(
                        ctxts[0:64, hp, 0:64], esb[0:64, hp, 0:64],
                        rsum[0:64, hp:hp + 1])
                    nc.vector.tensor_scalar_mul(
                        ctxts[64:128, hp, 64:128], esb[64:128, hp, 64:128],
                        rsum[64:128, hp:hp + 1])
                    tpc = pst.tile([128, CB, 128], F16, tag="tp",
                                   name=f"tpc{s}_{hp}")
                    nc.tensor.transpose(tpc[:, 0, :], ctxts[:, hp, :],
                                        ident16[:])
                    nc.scalar.copy(cbd[:, hp, :], tpc[:, 0, :])
                    if on_hp is not None:
                        on_hp(hp, cbd)
                    yield
                ctx_res[s] = cbd

            ctx_res = {}

            def out_mm(s, cb, g, cbd, ob):
                """o_s^T[cb block, group g] = cbd[cb] stationary @ xT."""
                op = pso.tile([128, C], F32, tag="op", name=f"op{s}_{cb}_{g}")
                nc.tensor.matmul(
                    op[:], lhsT=cbd[:, cb, :],
                    rhs=xts[s][:, cb, GW * g:GW * (g + 1)],
                    start=True, stop=True)
                eng[(cb + g) % 2](ob[:, cb, :], op[:])

            def store_o2(g, ob):
                nc.scalar.dma_start(
                    out=o_d[1][:, GW * g:GW * (g + 1)].rearrange(
                        "(a p) n -> p a n", p=128),
                    in_=ob[:])

            # ---- PE warm-up: dummy transposes bridge the DMA lead-in so
            # the p-state ramp (3us of continuous busy -> full clock) is
            # already progressing when the first gram matmul arrives ----
            wtp = pst.tile([128, CB, 128], F16, tag="tp", name="wtp")
            for _ in range(14):
                nc.tensor.transpose(wtp[:, 0, :], ident16[:], ident16[:])
            nc.vector.tensor_copy(ident16[:], wtp[:, 0, :])

            # ================= phase 1: x1 gram + xpose =================
            gps1 = psg.tile([128, GPW], F32, tag="gp", name="gp_0")
            for t in range(NT):
                gram_mm(0, t, gps1)
                xpose(0, t)
                if t == 16:
                    tt_weights(0)
                if t == 20:
                    tt_weights(1)

            # ========== phase 2: x2 gram + xpose + fused o2^T,  ==========
            # ========== interleaved with ctx_tail(1)'s stages   ==========
            gps2 = psg.tile([128, GPW], F32, tag="gp", name="gp_1")
            tail1 = ctx_tail_gen(0, gps1)
            next(tail1)          # emit G1 copies first (frees gram psum banks)
            t2 = 0               # phase-2 tile cursor

            def emit_tiles(k):
                nonlocal t2
                for _ in range(k):
                    if t2 >= NT:
                        return
                    gram_mm(1, t2, gps2)
                    xpose(1, t2)
                    t2 += 1

            while True:
                emit_tiles(2)
                try:
                    next(tail1)
                except StopIteration:
                    break
            # drain remaining x2 tiles, interleaving o2 groups as their
            # tiles (and cbd1) become available
            obs2 = {}
            for g in range(NG):
                emit_tiles(TPG * (g + 1) - t2)
                if g < NG - DEFER:
                    ob = osp.tile([128, CB, GW], F16, tag="ob", name=f"ob2_{g}")
                    for cb in range(CB):
                        out_mm(1, cb, g, ctx_res[0], ob)
                    store_o2(g, ob)

            # ==== tail(2) + phase 3, with deferred o2 groups as filler ====
            ob1s = [ob1p.tile([128, CB, GW], F16, tag="ob1", name=f"ob1_{g}")
                    for g in range(NG)]
            defer_q = [(cb, g) for g in range(NG - DEFER, NG) for cb in range(CB)]
            for g in range(NG - DEFER, NG):
                obs2[g] = osp.tile([128, CB, GW], F16, tag="ob",
                                   name=f"ob2_{g}")
            di = 0

            def emit_defer(k):
                nonlocal di
                for _ in range(k):
                    if di >= len(defer_q):
                        return
                    cb, g = defer_q[di]
                    out_mm(1, cb, g, ctx_res[0], obs2[g])
                    if cb == CB - 1:
                        store_o2(g, obs2[g])
                    di += 1

            def phase3(hp, cbd):
                for g in range(NG):
                    out_mm(0, hp, g, cbd, ob1s[g])
                for g in range(NG):
                    nc.sync.dma_start(
                        out=o_d[0][128 * hp:128 * (hp + 1),
                                   GW * g:GW * (g + 1)],
                        in_=ob1s[g][:, hp, :])

            tail2 = ctx_tail_gen(1, gps2, on_hp=phase3)
            next(tail2)
            while True:
                emit_defer(2)
                try:
                    next(tail2)
                except StopIteration:
                    break
            emit_defer(len(defer_q))
    nc.compile()
    return nc


_NC = None


def make_in_maps(inputs):
    x1 = np.asarray(inputs["x1"])
    x2 = np.asarray(inputs["x2"])
    w1 = np.ascontiguousarray(np.asarray(inputs["W_kv1"]), dtype=np.float16)
    w2 = np.ascontiguousarray(np.asarray(inputs["W_kv2"]), dtype=np.float16)
    in_maps = []
    for b in range(B):
        in_maps.append({
            "x1": np.ascontiguousarray(x1[b], dtype=np.float16),
            "x2": np.ascontiguousarray(x2[b], dtype=np.float16),
            "w1": w1, "w2": w2,
        })
    return in_maps


def kernel(x1, x2, W_kv1, W_kv2):
    global _NC
    if _NC is None:
        _NC = build()
    in_maps = make_in_maps(
        {"x1": x1, "x2": x2, "W_kv1": W_kv1, "W_kv2": W_kv2})
    res = run_bass_kernel_spmd(_NC, in_maps, core_ids=list(range(B)))
    o1 = np.stack([res.results[b]["ot1"].astype(np.float32).T
                   for b in range(B)])
    o2 = np.stack([res.results[b]["ot2"].astype(np.float32).T
                   for b in range(B)])
    return o1, o2


# revision 26
# speedup vs baseline: 1.1615x; 1.0598x over previous
"""Trainium2 Bass kernel for nn_CrossAttention_249108103802.

8 cores data-parallel over B=8; per core (batch b):
  G_s   = x_s^T x_s            (Gram, fp16 operands, fp32 psum, upper tri)
  A_s   = (G_s - mu I) Wv_s    (fp16)
  ctp_s = A_s^T Wk_s + mu Wv_s^T Wk_s   (fp16 pair-packed + fp32 TT)
  ctx_s = softmax_d(scale * ctp_s)      (per-head 64x64)
  o2^T  = blockdiag(ctx1) stationary @ xT2   fused into x2 streaming
  o1^T  = same with ctx2 / resident xT1

The PE p-state model rewards gap-free streams (half clock for 3us after
any idle), so the softmax tails are emitted as staged generators with
matmul filler interleaved between stages: phase-2 gram tiles fill
ctx_tail(1)'s stages, deferred o2 out-matmuls + per-head-pair o1
matmuls fill ctx_tail(2)'s. Host supplies x and W in fp16; xT is built
on-chip with PE transposes. Outputs are written as o^T [C, N] fp16 and
transposed back on the host.
"""
import sys

sys.path.insert(0, "/opt/trn_rl_repo")

import numpy as np

import concourse.bass as bass
import concourse.mybir as mybir
import concourse.tile as tile
from concourse import bacc
from concourse.bass_utils import run_bass_kernel_spmd
from concourse.masks import make_identity

B, N, C, H = 8, 4096, 512, 8
HD = C // H                    # 64
SCALE = HD ** -0.5             # 1/8
MU = float(N)
NT = N // 128                  # 32 row tiles
CB = C // 128                  # 4 feature blocks
HP = H // 2                    # 4 head pairs
NG = 8                         # streaming groups
GW = N // NG                   # 512 rows/cols per group
TPG = NT // NG                 # 4 row tiles per group
DEFER = 4                      # o2 groups deferred into ctx_tail(2)
F16 = mybir.dt.float16
F32 = mybir.dt.float32
AF = mybir.ActivationFunctionType

# Gram psum column ranges per row-block m (strict upper triangle)
GCOL = [(0, 512), (128, 512), (256, 512), (384, 512)]
# column offset of each m's accumulator inside the packed 3-bank psum
# tile: m1 (384 cols) and m3 (128 cols) share bank 1. m1 owns the bank's
# start (first write at t=0) and stop (last write at t=NT-1); m3 always
# runs with start=stop=False, relying on the bank's pending-zero bytes.
GOFF = [0, 512, 1024, 896]
GPW = 1536
# lower-triangle tiles needing a PE transpose
LOWT = [(1, 0), (2, 0), (2, 1), (3, 0), (3, 1), (3, 2)]


def build():
    nc = bacc.Bacc("TRN2", target_bir_lowering=False, debug=False, num_devices=8)
    x_d = [nc.declare_dram_parameter(f"x{s + 1}", [N, C], F16, isOutput=False)
           for s in range(2)]
    w_d = [nc.declare_dram_parameter(f"w{s + 1}", [C, 2 * C], F16, isOutput=False)
           for s in range(2)]
    o_d = [nc.declare_dram_parameter(f"ot{s + 1}", [C, N], F16, isOutput=True)
           for s in range(2)]

    with tile.TileContext(nc) as tc:
        with (
            tc.tile_pool(name="const", bufs=1) as constp,
            tc.tile_pool(name="wf", bufs=1) as wfp,
            tc.tile_pool(name="tts", bufs=1) as ttsp,
            tc.tile_pool(name="x", bufs=6) as xp,
            tc.tile_pool(name="xt", bufs=1) as xtp,
            tc.tile_pool(name="g", bufs=1) as gp_,
            tc.tile_pool(name="a", bufs=1) as ap_,
            tc.tile_pool(name="cx", bufs=1) as cxp,
            tc.tile_pool(name="osb", bufs=4) as osp,
            tc.tile_pool(name="ob1", bufs=HP) as ob1p,
            tc.tile_pool(name="ps_g", bufs=1, space="PSUM") as psg,
            tc.tile_pool(name="ps_t", bufs=2, space="PSUM") as pst,
            tc.tile_pool(name="ps_o", bufs=3, space="PSUM") as pso,
        ):
            identf = constp.tile([128, 128], F32, tag="identf")
            make_identity(nc, identf[:])
            ident16 = constp.tile([128, 128], F16, tag="ident16")
            nc.scalar.copy(ident16[:], identf[:])
            muI = constp.tile([128, 128], F32, tag="muI")
            nc.gpsimd.memset(muI[:], 0.0)
            nc.gpsimd.affine_select(
                out=muI[:], in_=muI[:],
                compare_op=mybir.AluOpType.not_equal, fill=MU,
                base=0, pattern=[[-1, 128]], channel_multiplier=1,
            )

            # ---- x1 streaming loads first (sync HWDGE queue; the DMA pipe
            # is serialized, so issue order is transfer order). First group
            # split in half so the very first gram matmul starts sooner ----
            xcs = {}
            for g in range(NG):
                xc = xp.tile([128, TPG, C], F16, tag="xc", name=f"xc0_{g}")
                if g == 0:
                    for h in range(2):
                        nc.sync.dma_start(
                            out=xc[:, 2 * h:2 * h + 2, :],
                            in_=x_d[0][256 * h:256 * (h + 1), :].rearrange(
                                "(t p) c -> p t c", p=128))
                else:
                    nc.sync.dma_start(
                        out=xc[:],
                        in_=x_d[0][GW * g:GW * (g + 1), :].rearrange(
                            "(t p) c -> p t c", p=128))
                xcs[(0, g)] = xc

            # ---- weight loads (same sync ring so they queue after x1) ----
            wfs, ttss = [], []
            for s in range(2):
                wf = wfp.tile([128, CB, 2 * C], F16, tag=f"wf{s}")
                nc.sync.dma_start(
                    out=wf[:], in_=w_d[s][:, :].rearrange("(a p) m -> p a m", p=128))
                wfs.append(wf)
                ttss.append(ttsp.tile([128, HP, 128], F32, tag=f"tts{s}",
                                      name=f"tts{s}"))

            # ---- x2 streaming loads ----
            for g in range(NG):
                xc = xp.tile([128, TPG, C], F16, tag="xc", name=f"xc1_{g}")
                nc.sync.dma_start(
                    out=xc[:],
                    in_=x_d[1][GW * g:GW * (g + 1), :].rearrange(
                        "(t p) c -> p t c", p=128))
                xcs[(1, g)] = xc

            xts = [xtp.tile([128, CB, N], F16, tag=f"xt{s}", name=f"xt{s}")
                   for s in range(2)]
            eng = [nc.vector.tensor_copy, nc.scalar.copy]

            def tt_weights(s):
                # exact TT = mu * Wv^T Wk, pair-packed [e(2h), d(2h)]
                wf = wfs[s]
                for hp in range(HP):
                    ttp = pso.tile([128, C], F32, tag="op", name=f"ttp{s}_{hp}")
                    for a in range(CB):
                        nc.tensor.matmul(
                            ttp[:, 0:128],
                            lhsT=wf[:, a, C + 128 * hp:C + 128 * (hp + 1)],
                            rhs=wf[:, a, 128 * hp:128 * (hp + 1)],
                            start=(a == 0), stop=(a == CB - 1))
                    nc.scalar.mul(ttss[s][:, hp, :], ttp[:, 0:128], MU)

            def gram_mm(s, t, gp):
                xc = xcs[(s, t // TPG)]
                tt_ = t % TPG
                order = [3, 1, 0, 2] if t == NT - 1 else [1, 3, 0, 2]
                for m in order:
                    lo, hi = GCOL[m]
                    nc.tensor.matmul(
                        gp[:, GOFF[m]:GOFF[m] + hi - lo],
                        lhsT=xc[:, tt_, 128 * m:128 * (m + 1)],
                        rhs=xc[:, tt_, lo:hi],
                        start=(t == 0 and m != 3),
                        stop=(t == NT - 1 and m != 3),
                        skip_group_check=(m == 3))

            def xpose(s, t):
                """PE-transpose the 4 column blocks of x tile t into xT (f16)."""
                xc = xcs[(s, t // TPG)]
                tt_ = t % TPG
                tp4 = pst.tile([128, CB, 128], F16, tag="tp", name=f"tp4_{s}_{t}")
                for cb in range(CB):
                    nc.tensor.transpose(
                        tp4[:, cb, :], xc[:, tt_, 128 * cb:128 * (cb + 1)],
                        ident16[:])
                eng[t % 2](xts[s][:, :, 128 * t:128 * (t + 1)], tp4[:])

            def ctx_tail_gen(s, gp, on_hp=None):
                """Staged G->A->ctp->softmax->cbd; yields between stages so
                the caller can interleave PE filler work."""
                wf = wfs[s]
                gsb = gp_.tile([128, CB, C], F16, tag="gsb", name=f"gsb{s}")
                for m in range(CB):
                    lo, hi = GCOL[m]
                    dg = GOFF[m] + 128 * m - lo  # diag offset in packed psum
                    nc.vector.tensor_sub(
                        gsb[:, m, 128 * m:128 * (m + 1)],
                        gp[:, dg:dg + 128], muI[:])
                    if m < 3:
                        eng[m % 2](gsb[:, m, 128 * (m + 1):C],
                                   gp[:, dg + 128:GOFF[m] + hi - lo])
                esb = cxp.tile([128, HP, 128], F32, tag="esb", name=f"esb{s}")
                ssum = cxp.tile([128, HP], F32, tag="ssum", name=f"ssum{s}")
                rsum = cxp.tile([128, HP], F32, tag="rsum", name=f"rsum{s}")
                comb = cxp.tile([128, HP, 128], F32, tag="comb", name=f"comb{s}")
                ctxts = cxp.tile([128, HP, 128], F16, tag="ctxts",
                                 name=f"ctxts{s}")
                nc.gpsimd.memset(ctxts[:], 0.0)
                yield
                # lower-triangle tiles by PE transpose (f16)
                gtr = gp_.tile([128, len(LOWT), 128], F16, tag="gtr",
                               name=f"gtr{s}")
                for i, (a2, b2) in enumerate(LOWT):
                    tpg = pst.tile([128, CB, 128], F16, tag="tp",
                                   name=f"tpg{s}_{i}")
                    nc.tensor.transpose(
                        tpg[:, 0, :], gsb[:, b2, 128 * a2:128 * (a2 + 1)],
                        ident16[:])
                    nc.vector.tensor_copy(gtr[:, i, :], tpg[:, 0, :])
                low = {ab_: i for i, ab_ in enumerate(LOWT)}
                yield

                def g_tile(a2, b2):
                    if b2 >= a2:
                        return gsb[:, a2, 128 * b2:128 * (b2 + 1)]
                    return gtr[:, low[(a2, b2)], :]

                # A = Gc^T-tiles @ Wv (f16, free 512)
                ab = ap_.tile([128, CB, C], F16, tag="ab", name=f"ab{s}")
                for b2 in range(CB):
                    apx = pso.tile([128, C], F32, tag="op", name=f"apx{s}_{b2}")
                    for a2 in range(CB):
                        nc.tensor.matmul(
                            apx[:], lhsT=g_tile(a2, b2), rhs=wf[:, a2, C:2 * C],
                            start=(a2 == 0), stop=(a2 == CB - 1))
                    eng[b2 % 2](ab[:, b2, :], apx[:])
                    if b2 == 1:
                        yield
                yield
                # ctp (pair-packed) + TT, exp halves, per-hp reciprocal
                for hp in range(HP):
                    ctp = pso.tile([128, C], F32, tag="op", name=f"ctp{s}_{hp}")
                    sl = slice(128 * hp, 128 * (hp + 1))
                    for b2 in range(CB):
                        nc.tensor.matmul(
                            ctp[:, 0:128], lhsT=ab[:, b2, sl], rhs=wf[:, b2, sl],
                            start=(b2 == 0), stop=(b2 == CB - 1))
                    nc.vector.tensor_add(comb[:, hp, :], ctp[:, 0:128],
                                         ttss[s][:, hp, :])
                    nc.scalar.activation(
                        esb[0:64, hp, 0:64], comb[0:64, hp, 0:64], AF.Exp,
                        scale=SCALE, accum_out=ssum[0:64, hp:hp + 1])
                    nc.scalar.activation(
                        esb[64:128, hp, 64:128], comb[64:128, hp, 64:128], AF.Exp,
                        scale=SCALE, accum_out=ssum[64:128, hp:hp + 1])
                    nc.vector.reciprocal(rsum[:, hp:hp + 1], ssum[:, hp:hp + 1])
                    if hp % 2 == 1:
                        yield
                cbd = cxp.tile([128, HP, 128], F16, tag=f"cbd{s}")
                for hp in range(HP):
                    nc.vector.tensor_scalar_mul(
                        ctxts[0:64, hp, 0:64], esb[0:64, hp, 0:64],
                        rsum[0:64, hp:hp + 1])
                    nc.vector.tensor_scalar_mul(
                        ctxts[64:128, hp, 64:128], esb[64:128, hp, 64:128],
                        rsum[64:128, hp:hp + 1])
                    tpc = pst.tile([128, CB, 128], F16, tag="tp",
                                   name=f"tpc{s}_{hp}")
                    nc.tensor.transpose(tpc[:, 0, :], ctxts[:, hp, :],
                                        ident16[:])
                    nc.scalar.copy(cbd[:, hp, :], tpc[:, 0, :])
                    if on_hp is not None:
                        on_hp(hp, cbd)
                    yield
                ctx_res[s] = cbd

            ctx_res = {}

            def out_mm(s, cb, g, cbd, ob):
                """o_s^T[cb block, group g] = cbd[cb] stationary @ xT."""
                op = pso.tile([128, C], F32, tag="op", name=f"op{s}_{cb}_{g}")
                nc.tensor.matmul(
                    op[:], lhsT=cbd[:, cb, :],
                    rhs=xts[s][:, cb, GW * g:GW * (g + 1)],
                    start=True, stop=True)
                eng[(cb + g) % 2](ob[:, cb, :], op[:])

            def store_o2(g, ob):
                nc.scalar.dma_start(
                    out=o_d[1][:, GW * g:GW * (g + 1)].rearrange(
                        "(a p) n -> p a n", p=128),
                    in_=ob[:])

            # ---- PE warm-up: dummy transposes bridge the DMA lead-in so
            # the p-state ramp (3us of continuous busy -> full clock) is
            # already progressing when the first gram matmul arrives ----
            wtp = pst.tile([128, CB, 128], F16, tag="tp", name="wtp")
            for _ in range(14):
                nc.tensor.transpose(wtp[:, 0, :], ident16[:], ident16[:])
            nc.vector.tensor_copy(ident16[:], wtp[:, 0, :])

            # ================= phase 1: x1 gram + xpose =================
            gps1 = psg.tile([128, GPW], F32, tag="gp", name="gp_0")
            for t in range(NT):
                gram_mm(0, t, gps1)
                xpose(0, t)
                if t == 16:
                    tt_weights(0)
                if t == 20:
                    tt_weights(1)

            # ========== phase 2: x2 gram + xpose + fused o2^T,  ==========
            # ========== interleaved with ctx_tail(1)'s stages   ==========
            gps2 = psg.tile([128, GPW], F32, tag="gp", name="gp_1")
            tail1 = ctx_tail_gen(0, gps1)
            next(tail1)          # emit G1 copies first (frees gram psum banks)
            t2 = 0               # phase-2 tile cursor

            def emit_tiles(k):
                nonlocal t2
                for _ in range(k):
                    if t2 >= NT:
                        return
                    gram_mm(1, t2, gps2)
                    xpose(1, t2)
                    t2 += 1

            while True:
                emit_tiles(2)
                try:
                    next(tail1)
                except StopIteration:
                    break
            # drain remaining x2 tiles, interleaving o2 groups as their
            # tiles (and cbd1) become available
            obs2 = {}
            for g in range(NG):
                emit_tiles(TPG * (g + 1) - t2)
                if g < NG - DEFER:
                    ob = osp.tile([128, CB, GW], F16, tag="ob", name=f"ob2_{g}")
                    for cb in range(CB):
                        out_mm(1, cb, g, ctx_res[0], ob)
                    store_o2(g, ob)

            # ==== tail(2) + phase 3, with deferred o2 groups as filler ====
            # o1 is staged per head-pair as [128, N] so each hp needs just
            # one big row-contiguous store
            ob1s = [ob1p.tile([128, N], F16, tag="ob1", name=f"ob1_{hp}")
                    for hp in range(HP)]
            defer_q = [(cb, g) for g in range(NG - DEFER, NG) for cb in range(CB)]
            for g in range(NG - DEFER, NG):
                obs2[g] = osp.tile([128, CB, GW], F16, tag="ob",
                                   name=f"ob2_{g}")
            di = 0

            def emit_defer(k):
                nonlocal di
                for _ in range(k):
                    if di >= len(defer_q):
                        return
                    cb, g = defer_q[di]
                    out_mm(1, cb, g, ctx_res[0], obs2[g])
                    if cb == CB - 1:
                        store_o2(g, obs2[g])
                    di += 1

            def phase3(hp, cbd):
                for g in range(NG):
                    op = pso.tile([128, C], F32, tag="op", name=f"op3_{hp}_{g}")
                    nc.tensor.matmul(
                        op[:], lhsT=cbd[:, hp, :],
                        rhs=xts[0][:, hp, GW * g:GW * (g + 1)],
                        start=True, stop=True)
                    eng[g % 2](ob1s[hp][:, GW * g:GW * (g + 1)], op[:])
                nc.scalar.dma_start(
                    out=o_d[0][128 * hp:128 * (hp + 1), :],
                    in_=ob1s[hp][:])

            tail2 = ctx_tail_gen(1, gps2, on_hp=phase3)
            next(tail2)
            while True:
                emit_defer(2)
                try:
                    next(tail2)
                except StopIteration:
                    break
            emit_defer(len(defer_q))
    nc.compile()
    return nc


_NC = None


def make_in_maps(inputs):
    x1 = np.asarray(inputs["x1"])
    x2 = np.asarray(inputs["x2"])
    w1 = np.ascontiguousarray(np.asarray(inputs["W_kv1"]), dtype=np.float16)
    w2 = np.ascontiguousarray(np.asarray(inputs["W_kv2"]), dtype=np.float16)
    in_maps = []
    for b in range(B):
        in_maps.append({
            "x1": np.ascontiguousarray(x1[b], dtype=np.float16),
            "x2": np.ascontiguousarray(x2[b], dtype=np.float16),
            "w1": w1, "w2": w2,
        })
    return in_maps


def kernel(x1, x2, W_kv1, W_kv2):
    global _NC
    if _NC is None:
        _NC = build()
    in_maps = make_in_maps(
        {"x1": x1, "x2": x2, "W_kv1": W_kv1, "W_kv2": W_kv2})
    res = run_bass_kernel_spmd(_NC, in_maps, core_ids=list(range(B)))
    o1 = np.stack([res.results[b]["ot1"].astype(np.float32).T
                   for b in range(B)])
    o2 = np.stack([res.results[b]["ot2"].astype(np.float32).T
                   for b in range(B)])
    return o1, o2


# revision 29
# speedup vs baseline: 1.1730x; 1.0099x over previous
"""Trainium2 Bass kernel for nn_CrossAttention_249108103802.

8 cores data-parallel over B=8; per core (batch b):
  G_s   = x_s^T x_s            (Gram, fp16 operands, fp32 psum, upper tri)
  A_s   = (G_s - mu I) Wv_s    (fp16)
  ctp_s = A_s^T Wk_s + mu Wv_s^T Wk_s   (fp16 pair-packed + fp32 TT)
  ctx_s = softmax_d(scale * ctp_s)      (per-head 64x64)
  o2^T  = blockdiag(ctx1) stationary @ xT2   fused into x2 streaming
  o1^T  = same with ctx2 / resident xT1

The PE p-state model rewards gap-free streams (half clock for 3us after
any idle), so the softmax tails are emitted as staged generators with
matmul filler interleaved between stages: phase-2 gram tiles fill
ctx_tail(1)'s stages, deferred o2 out-matmuls + per-head-pair o1
matmuls fill ctx_tail(2)'s. Host supplies x and W in fp16; xT is built
on-chip with PE transposes. Outputs are written as o^T [C, N] fp16 and
transposed back on the host.
"""
import sys

sys.path.insert(0, "/opt/trn_rl_repo")

import numpy as np

import concourse.bass as bass
import concourse.mybir as mybir
import concourse.tile as tile
from concourse import bacc
from concourse.bass_utils import run_bass_kernel_spmd
from concourse.masks import make_identity

B, N, C, H = 8, 4096, 512, 8
HD = C // H                    # 64
SCALE = HD ** -0.5             # 1/8
MU = float(N)
NT = N // 128                  # 32 row tiles
CB = C // 128                  # 4 feature blocks
HP = H // 2                    # 4 head pairs
NG = 8                         # streaming groups
GW = N // NG                   # 512 rows/cols per group
TPG = NT // NG                 # 4 row tiles per group
DEFER = 4                      # o2 groups deferred into ctx_tail(2)
F16 = mybir.dt.float16
F32 = mybir.dt.float32
AF = mybir.ActivationFunctionType

# Gram psum column ranges per row-block m (strict upper triangle)
GCOL = [(0, 512), (128, 512), (256, 512), (384, 512)]
# column offset of each m's accumulator inside the packed 3-bank psum
# tile: m1 (384 cols) and m3 (128 cols) share bank 1. m1 owns the bank's
# start (first write at t=0) and stop (last write at t=NT-1); m3 always
# runs with start=stop=False, relying on the bank's pending-zero bytes.
GOFF = [0, 512, 1024, 896]
GPW = 1536
# lower-triangle tiles needing a PE transpose
LOWT = [(1, 0), (2, 0), (2, 1), (3, 0), (3, 1), (3, 2)]


def build():
    nc = bacc.Bacc("TRN2", target_bir_lowering=False, debug=False, num_devices=8)
    x_d = [nc.declare_dram_parameter(f"x{s + 1}", [N, C], F16, isOutput=False)
           for s in range(2)]
    w_d = [nc.declare_dram_parameter(f"w{s + 1}", [C, 2 * C], F16, isOutput=False)
           for s in range(2)]
    o_d = [nc.declare_dram_parameter(f"ot{s + 1}", [C, N], F16, isOutput=True)
           for s in range(2)]

    with tile.TileContext(nc) as tc:
        with (
            tc.tile_pool(name="const", bufs=1) as constp,
            tc.tile_pool(name="wf", bufs=1) as wfp,
            tc.tile_pool(name="tts", bufs=1) as ttsp,
            tc.tile_pool(name="x", bufs=6) as xp,
            tc.tile_pool(name="xt", bufs=1) as xtp,
            tc.tile_pool(name="g", bufs=1) as gp_,
            tc.tile_pool(name="a", bufs=1) as ap_,
            tc.tile_pool(name="cx", bufs=1) as cxp,
            tc.tile_pool(name="osb", bufs=4) as osp,
            tc.tile_pool(name="ob1", bufs=HP) as ob1p,
            tc.tile_pool(name="ps_g", bufs=1, space="PSUM") as psg,
            tc.tile_pool(name="ps_t", bufs=2, space="PSUM") as pst,
            tc.tile_pool(name="ps_o", bufs=3, space="PSUM") as pso,
        ):
            identf = constp.tile([128, 128], F32, tag="identf")
            make_identity(nc, identf[:])
            ident16 = constp.tile([128, 128], F16, tag="ident16")
            nc.scalar.copy(ident16[:], identf[:])
            muI = constp.tile([128, 128], F32, tag="muI")
            nc.gpsimd.memset(muI[:], 0.0)
            nc.gpsimd.affine_select(
                out=muI[:], in_=muI[:],
                compare_op=mybir.AluOpType.not_equal, fill=MU,
                base=0, pattern=[[-1, 128]], channel_multiplier=1,
            )

            # ---- x1 streaming loads first (sync HWDGE queue; the DMA pipe
            # is serialized, so issue order is transfer order). First group
            # split in half so the very first gram matmul starts sooner ----
            xcs = {}
            for g in range(NG):
                xc = xp.tile([128, TPG, C], F16, tag="xc", name=f"xc0_{g}")
                if g == 0:
                    for h in range(2):
                        nc.sync.dma_start(
                            out=xc[:, 2 * h:2 * h + 2, :],
                            in_=x_d[0][256 * h:256 * (h + 1), :].rearrange(
                                "(t p) c -> p t c", p=128))
                else:
                    nc.sync.dma_start(
                        out=xc[:],
                        in_=x_d[0][GW * g:GW * (g + 1), :].rearrange(
                            "(t p) c -> p t c", p=128))
                xcs[(0, g)] = xc

            # ---- weight loads (same sync ring so they queue after x1) ----
            wfs, ttss = [], []
            for s in range(2):
                wf = wfp.tile([128, CB, 2 * C], F16, tag=f"wf{s}")
                nc.sync.dma_start(
                    out=wf[:], in_=w_d[s][:, :].rearrange("(a p) m -> p a m", p=128))
                wfs.append(wf)
                ttss.append(ttsp.tile([128, HP, 128], F32, tag=f"tts{s}",
                                      name=f"tts{s}"))

            # ---- x2 streaming loads ----
            for g in range(NG):
                xc = xp.tile([128, TPG, C], F16, tag="xc", name=f"xc1_{g}")
                nc.sync.dma_start(
                    out=xc[:],
                    in_=x_d[1][GW * g:GW * (g + 1), :].rearrange(
                        "(t p) c -> p t c", p=128))
                xcs[(1, g)] = xc

            xts = [xtp.tile([128, CB, N], F16, tag=f"xt{s}", name=f"xt{s}")
                   for s in range(2)]
            eng = [nc.vector.tensor_copy, nc.scalar.copy]

            def tt_weights(s):
                # exact TT = mu * Wv^T Wk, pair-packed [e(2h), d(2h)]
                wf = wfs[s]
                for hp in range(HP):
                    ttp = pso.tile([128, C], F32, tag="op", name=f"ttp{s}_{hp}")
                    for a in range(CB):
                        nc.tensor.matmul(
                            ttp[:, 0:128],
                            lhsT=wf[:, a, C + 128 * hp:C + 128 * (hp + 1)],
                            rhs=wf[:, a, 128 * hp:128 * (hp + 1)],
                            start=(a == 0), stop=(a == CB - 1))
                    nc.scalar.mul(ttss[s][:, hp, :], ttp[:, 0:128], MU)

            def gram_mm(s, t, gp):
                xc = xcs[(s, t // TPG)]
                tt_ = t % TPG
                order = [3, 1, 0, 2] if t == NT - 1 else [1, 3, 0, 2]
                for m in order:
                    lo, hi = GCOL[m]
                    nc.tensor.matmul(
                        gp[:, GOFF[m]:GOFF[m] + hi - lo],
                        lhsT=xc[:, tt_, 128 * m:128 * (m + 1)],
                        rhs=xc[:, tt_, lo:hi],
                        start=(t == 0 and m != 3),
                        stop=(t == NT - 1 and m != 3),
                        skip_group_check=(m == 3))

            def xpose(s, t):
                """PE-transpose the 4 column blocks of x tile t into xT (f16)."""
                xc = xcs[(s, t // TPG)]
                tt_ = t % TPG
                tp4 = pst.tile([128, CB, 128], F16, tag="tp", name=f"tp4_{s}_{t}")
                for cb in range(CB):
                    nc.tensor.transpose(
                        tp4[:, cb, :], xc[:, tt_, 128 * cb:128 * (cb + 1)],
                        ident16[:])
                eng[t % 2](xts[s][:, :, 128 * t:128 * (t + 1)], tp4[:])

            def ctx_tail_gen(s, gp, on_hp=None):
                """Staged G->A->ctp->softmax->cbd; yields between stages so
                the caller can interleave PE filler work."""
                wf = wfs[s]
                gsb = gp_.tile([128, CB, C], F16, tag="gsb", name=f"gsb{s}")
                for m in range(CB):
                    lo, hi = GCOL[m]
                    dg = GOFF[m] + 128 * m - lo  # diag offset in packed psum
                    nc.vector.tensor_sub(
                        gsb[:, m, 128 * m:128 * (m + 1)],
                        gp[:, dg:dg + 128], muI[:])
                    if m < 3:
                        eng[m % 2](gsb[:, m, 128 * (m + 1):C],
                                   gp[:, dg + 128:GOFF[m] + hi - lo])
                esb = cxp.tile([128, HP, 128], F32, tag="esb", name=f"esb{s}")
                ssum = cxp.tile([128, HP], F32, tag="ssum", name=f"ssum{s}")
                rsum = cxp.tile([128, HP], F32, tag="rsum", name=f"rsum{s}")
                comb = cxp.tile([128, HP, 128], F32, tag="comb", name=f"comb{s}")
                ctxts = cxp.tile([128, HP, 128], F16, tag="ctxts",
                                 name=f"ctxts{s}")
                nc.gpsimd.memset(ctxts[:], 0.0)
                yield
                # lower-triangle tiles by PE transpose (f16)
                gtr = gp_.tile([128, len(LOWT), 128], F16, tag="gtr",
                               name=f"gtr{s}")
                for i, (a2, b2) in enumerate(LOWT):
                    tpg = pst.tile([128, CB, 128], F16, tag="tp",
                                   name=f"tpg{s}_{i}")
                    nc.tensor.transpose(
                        tpg[:, 0, :], gsb[:, b2, 128 * a2:128 * (a2 + 1)],
                        ident16[:])
                    nc.vector.tensor_copy(gtr[:, i, :], tpg[:, 0, :])
                low = {ab_: i for i, ab_ in enumerate(LOWT)}
                yield

                def g_tile(a2, b2):
                    if b2 >= a2:
                        return gsb[:, a2, 128 * b2:128 * (b2 + 1)]
                    return gtr[:, low[(a2, b2)], :]

                # A = Gc^T-tiles @ Wv (f16, free 512)
                ab = ap_.tile([128, CB, C], F16, tag="ab", name=f"ab{s}")
                for b2 in range(CB):
                    apx = pso.tile([128, C], F32, tag="op", name=f"apx{s}_{b2}")
                    for a2 in range(CB):
                        nc.tensor.matmul(
                            apx[:], lhsT=g_tile(a2, b2), rhs=wf[:, a2, C:2 * C],
                            start=(a2 == 0), stop=(a2 == CB - 1))
                    eng[b2 % 2](ab[:, b2, :], apx[:])
                    if b2 == 1:
                        yield
                yield
                # ctp (pair-packed) + TT, exp halves, per-hp reciprocal
                for hp in range(HP):
                    ctp = pso.tile([128, C], F32, tag="op", name=f"ctp{s}_{hp}")
                    sl = slice(128 * hp, 128 * (hp + 1))
                    for b2 in range(CB):
                        nc.tensor.matmul(
                            ctp[:, 0:128], lhsT=ab[:, b2, sl], rhs=wf[:, b2, sl],
                            start=(b2 == 0), stop=(b2 == CB - 1))
                    nc.vector.tensor_add(comb[:, hp, :], ctp[:, 0:128],
                                         ttss[s][:, hp, :])
                    nc.scalar.activation(
                        esb[0:64, hp, 0:64], comb[0:64, hp, 0:64], AF.Exp,
                        scale=SCALE, accum_out=ssum[0:64, hp:hp + 1])
                    nc.scalar.activation(
                        esb[64:128, hp, 64:128], comb[64:128, hp, 64:128], AF.Exp,
                        scale=SCALE, accum_out=ssum[64:128, hp:hp + 1])
                    nc.vector.reciprocal(rsum[:, hp:hp + 1], ssum[:, hp:hp + 1])
                    if hp % 2 == 1:
                        yield
                # all normalize/transpose chains up front (V/S are still
                # idle here) so the big out-matmul stream that follows
                # never waits on them
                cbd = cxp.tile([128, HP, 128], F16, tag=f"cbd{s}")
                for hp in range(HP):
                    nc.vector.tensor_scalar_mul(
                        ctxts[0:64, hp, 0:64], esb[0:64, hp, 0:64],
                        rsum[0:64, hp:hp + 1])
                    nc.vector.tensor_scalar_mul(
                        ctxts[64:128, hp, 64:128], esb[64:128, hp, 64:128],
                        rsum[64:128, hp:hp + 1])
                    tpc = pst.tile([128, CB, 128], F16, tag="tp",
                                   name=f"tpc{s}_{hp}")
                    nc.tensor.transpose(tpc[:, 0, :], ctxts[:, hp, :],
                                        ident16[:])
                    nc.scalar.copy(cbd[:, hp, :], tpc[:, 0, :])
                ctx_res[s] = cbd

            ctx_res = {}

            def out_mm(s, cb, g, cbd, ob):
                """o_s^T[cb block, group g] = cbd[cb] stationary @ xT."""
                op = pso.tile([128, C], F32, tag="op", name=f"op{s}_{cb}_{g}")
                nc.tensor.matmul(
                    op[:], lhsT=cbd[:, cb, :],
                    rhs=xts[s][:, cb, GW * g:GW * (g + 1)],
                    start=True, stop=True)
                eng[(cb + g) % 2](ob[:, cb, :], op[:])

            def store_o2(g, ob):
                nc.scalar.dma_start(
                    out=o_d[1][:, GW * g:GW * (g + 1)].rearrange(
                        "(a p) n -> p a n", p=128),
                    in_=ob[:])

            # ---- PE warm-up: dummy transposes bridge the DMA lead-in so
            # the p-state ramp (3us of continuous busy -> full clock) is
            # already progressing when the first gram matmul arrives ----
            wtp = pst.tile([128, CB, 128], F16, tag="tp", name="wtp")
            for _ in range(14):
                nc.tensor.transpose(wtp[:, 0, :], ident16[:], ident16[:])
            nc.vector.tensor_copy(ident16[:], wtp[:, 0, :])

            # ================= phase 1: x1 gram + xpose =================
            gps1 = psg.tile([128, GPW], F32, tag="gp", name="gp_0")
            for t in range(NT):
                gram_mm(0, t, gps1)
                xpose(0, t)
                if t == 16:
                    tt_weights(0)
                if t == 20:
                    tt_weights(1)

            # ========== phase 2: x2 gram + xpose + fused o2^T,  ==========
            # ========== interleaved with ctx_tail(1)'s stages   ==========
            gps2 = psg.tile([128, GPW], F32, tag="gp", name="gp_1")
            tail1 = ctx_tail_gen(0, gps1)
            next(tail1)          # emit G1 copies first (frees gram psum banks)
            t2 = 0               # phase-2 tile cursor

            def emit_tiles(k):
                nonlocal t2
                for _ in range(k):
                    if t2 >= NT:
                        return
                    gram_mm(1, t2, gps2)
                    xpose(1, t2)
                    t2 += 1

            for w in (3, 3, 2, 2, 2, 2, 2, 2, 2):
                emit_tiles(w)
                try:
                    next(tail1)
                except StopIteration:
                    break
            # drain remaining x2 tiles, interleaving inline o2 matmuls
            # one-per-tile as their groups (and cbd1) become available
            obs2 = {}
            pend = [(cb, g) for g in range(NG - DEFER) for cb in range(CB)]
            pi = 0

            def emit_pend(k):
                nonlocal pi
                while k > 0 and pi < len(pend):
                    cb, g = pend[pi]
                    if t2 < TPG * (g + 1):
                        return
                    if cb == 0:
                        obs2[g] = osp.tile([128, CB, GW], F16, tag="ob",
                                           name=f"ob2_{g}")
                    out_mm(1, cb, g, ctx_res[0], obs2[g])
                    if cb == CB - 1:
                        store_o2(g, obs2[g])
                    pi += 1
                    k -= 1

            while t2 < NT or pi < len(pend):
                emit_tiles(1)
                emit_pend(1)
                if t2 >= NT:
                    emit_pend(len(pend))

            # ==== tail(2) + phase 3, with deferred o2 groups as filler ====
            # o1 is staged per head-pair as [128, N] so each hp needs just
            # one big row-contiguous store
            ob1s = [ob1p.tile([128, N], F16, tag="ob1", name=f"ob1_{hp}")
                    for hp in range(HP)]
            defer_q = [(cb, g) for g in range(NG - DEFER, NG) for cb in range(CB)]
            for g in range(NG - DEFER, NG):
                obs2[g] = osp.tile([128, CB, GW], F16, tag="ob",
                                   name=f"ob2_{g}")
            di = 0

            def emit_defer(k):
                nonlocal di
                for _ in range(k):
                    if di >= len(defer_q):
                        return
                    cb, g = defer_q[di]
                    out_mm(1, cb, g, ctx_res[0], obs2[g])
                    if cb == CB - 1:
                        store_o2(g, obs2[g])
                    di += 1

            tail2 = ctx_tail_gen(1, gps2)
            next(tail2)
            for w in (5, 4, 3, 2, 1, 1, 1, 1, 1):
                emit_defer(w)
                try:
                    next(tail2)
                except StopIteration:
                    break

            # o1^T stream: 8 matmuls + one full-row store per head-pair,
            # with any remaining deferred o2 matmuls woven in
            cbd2 = ctx_res[1]
            for hp in range(HP):
                for g in range(NG):
                    op = pso.tile([128, C], F32, tag="op", name=f"op3_{hp}_{g}")
                    nc.tensor.matmul(
                        op[:], lhsT=cbd2[:, hp, :],
                        rhs=xts[0][:, hp, GW * g:GW * (g + 1)],
                        start=True, stop=True)
                    eng[g % 2](ob1s[hp][:, GW * g:GW * (g + 1)], op[:])
                nc.scalar.dma_start(
                    out=o_d[0][128 * hp:128 * (hp + 1), :],
                    in_=ob1s[hp][:])
                emit_defer(4)
            emit_defer(len(defer_q))
    nc.compile()
    return nc


_NC = None


def make_in_maps(inputs):
    x1 = np.asarray(inputs["x1"])
    x2 = np.asarray(inputs["x2"])
    w1 = np.ascontiguousarray(np.asarray(inputs["W_kv1"]), dtype=np.float16)
    w2 = np.ascontiguousarray(np.asarray(inputs["W_kv2"]), dtype=np.float16)
    in_maps = []
    for b in range(B):
        in_maps.append({
            "x1": np.ascontiguousarray(x1[b], dtype=np.float16),
            "x2": np.ascontiguousarray(x2[b], dtype=np.float16),
            "w1": w1, "w2": w2,
        })
    return in_maps


def kernel(x1, x2, W_kv1, W_kv2):
    global _NC
    if _NC is None:
        _NC = build()
    in_maps = make_in_maps(
        {"x1": x1, "x2": x2, "W_kv1": W_kv1, "W_kv2": W_kv2})
    res = run_bass_kernel_spmd(_NC, in_maps, core_ids=list(range(B)))
    o1 = np.stack([res.results[b]["ot1"].astype(np.float32).T
                   for b in range(B)])
    o2 = np.stack([res.results[b]["ot2"].astype(np.float32).T
                   for b in range(B)])
    return o1, o2
